# revision 5
# baseline (speedup 1.0000x reference)
"""TRN2 Bass kernel for nn_CAModule (cross-attention module).

Reference computation (per batch b):
    q = wq @ xq + bq            (128, Nq)
    k = wk @ xk + bk            (128, Nk)
    v = wv @ xk + bv            (128, Nk)
    e = q^T k                   (Nq, Nk)
    a = softmax(e, axis=-1)
    out = v @ a^T               (128, Nq)
    y = wo @ out + bo + xq      (256, Nq)

Sharding: 8 cores = 4 batches x 2 query-halves. Each core handles 2048
queries against all 4096 keys of its batch.

Math simplifications (exact under softmax):
  - bk drops out (adds a per-row constant to e; softmax-invariant)
  - bv folds into bo' = bo + wo @ bv (softmax rows sum to 1)
  - softmax computed without max subtraction (|e| <= ~20 -> exp safe in f32)

On-chip layout (per core):
  - projections + energy matmuls in fp32r (tf32-like, full PE rate at N>=256)
  - energy computed transposed: eT[k, q] = k^T q, exp'd on ACT into bf16
  - AV as out_T[q, c] = sum_k eT[k, :]^T vT[k, :] with a ones-column
    appended to vT so column 128 of the accumulator is the softmax
    denominator; normalization is then a per-partition ACT scale
  - PE transpose of out_T -> out[c, q], then output projection + residual
"""
import sys

sys.path.insert(0, "/opt/trn_rl_repo")

from contextlib import ExitStack

import numpy as np

import concourse.bass as bass
import concourse.tile as tile
from concourse import mybir
from concourse.bass_utils import run_bass_kernel_spmd
from concourse.masks import make_identity
from concourse.vector_clock import ScopedClock, VectorClock

F32 = mybir.dt.float32
F32R = mybir.dt.float32r
BF16 = mybir.dt.bfloat16
AF = mybir.ActivationFunctionType

P = 128          # partitions
CH = 128         # attention channels (C/2)
CIN = 256        # input channels
NG = CIN // P    # input-channel groups (2)
NK = 4096        # keys per batch
NQ = 2048        # queries per core
QC = 512         # query chunk (eT block width)
NCHUNK = NQ // QC
GRP = 4          # k-chunks per exp group
NKC = NK // P    # 32 k-chunks
NGRP = NKC // GRP
NQT = QC // P    # q-tiles per chunk

AVDT = BF16      # dtype of attention weights / v / out projection operands
NAV = CH + 1     # AV matmul stream width (v columns + ones column)
VTW = CH + 4     # vT tile width (pad a little)


def _split_drain_and_barrier(self, tick_clock, wait_clock):
    """Tail drain with one sem wait per instruction.

    The stock TileContext attaches every outstanding proc's wait to a single
    Drain, which the walrus codegen on this path rejects ("Too many sync
    wait commands"). Emit one drain per proc instead.
    """
    g = tick_clock.global_clock
    n = len(g)
    for p in range(n):
        if g[p] > 0:
            d = self.nc.sync.drain()
            pc = [0] * n
            pc[p] = g[p]
            wait_clock.add_sem_waits(d.ins, ScopedClock({None: VectorClock(pc)}))
    self.nc.all_engine_barrier()
    assert self.sems is not None
    popped = self.nc._tile_sem_poison_stack.pop()
    assert popped is self._sem_poison
    self.nc.clear_and_free_semaphores(list(self.sems.allocated().values()))
    self.nc.all_engine_barrier()


tile.TileContext._drain_and_barrier = _split_drain_and_barrier


def _split_multi_waits(nc):
    """Rewrite the scheduled program so no instruction carries more than one
    sync wait (the ISA has a single wait slot per instruction and this
    toolchain's codegen refuses to split them). Extra waits are hoisted onto
    engine NOPs inserted just before the instruction."""
    import bass_rust

    ctr = 0
    for f in nc.m.functions:
        for blk in f.blocks:
            out = []
            for inst in blk.instructions:
                si = inst.sync_info
                if si is not None and si.on_wait is not None and len(si.on_wait) > 1:
                    waits = list(si.on_wait)
                    for w in waits[:-1]:
                        nop = mybir.InstNoOp(name=f"Wnop-{ctr}", ins=[], outs=[])
                        ctr += 1
                        nop.engine = inst.engine
                        nop.sync_info = bass_rust.SyncInfo(
                            on_wait=[w], on_update=[]
                        )
                        out.append(nop)
                    inst.sync_info = bass_rust.SyncInfo(
                        on_wait=[waits[-1]], on_update=list(si.on_update or [])
                    )
                out.append(inst)
            blk.instructions = out
    return ctr


def _emit(nc, tc, ctx):
    persist = ctx.enter_context(tc.tile_pool(name="persist", bufs=1))

    # ---- persistent tiles ----
    xq_sb = persist.tile([P, NG, NQ], F32)          # becomes xq + bo' in place
    qr = persist.tile([P, NQ], F32R)
    kr = persist.tile([P, NK], F32R)
    vt = [
        persist.tile([P, VTW], AVDT, tag=f"vt{kc}", name=f"vt{kc}")
        for kc in range(NKC)
    ]
    y_sb = persist.tile([P, NG, NQ], F32)
    woT_bf = persist.tile([P, CIN], BF16)
    bq_sb = persist.tile([P, 1], F32)
    bo2_sb = persist.tile([P, NG], F32)
    ident = persist.tile([P, P], BF16, tag="ident")

    make_identity(nc, ident[:])
    for kc in range(NKC):
        nc.vector.memset(vt[kc][:, CH : CH + 1], 1.0)

    with tc.tile_pool(name="ph1", bufs=1) as ph1, tc.tile_pool(
        name="ph1ps", bufs=2, space="PSUM"
    ) as ph1ps:
        xq_r = ph1.tile([P, NG, NQ], F32R)
        xk_sb = ph1.tile([P, NG, NK], F32)
        xk_r = ph1.tile([P, NG, NK], F32R)
        xk_bf = ph1.tile([P, NG, NK], BF16)
        wqT_sb = ph1.tile([P, NG, CH], F32, tag="wq")
        wkT_sb = ph1.tile([P, NG, CH], F32, tag="wk")
        wvT_sb = ph1.tile([P, NG, CH], F32, tag="wv")
        woT_sb = ph1.tile([P, CIN], F32, tag="wo")
        wqT_r = ph1.tile([P, NG, CH], F32R, tag="wqr")
        wkT_r = ph1.tile([P, NG, CH], F32R, tag="wkr")
        wvT_bf = ph1.tile([P, NG, CH], BF16, tag="wvb")

        # ---- input DMAs ----
        nc.sync.dma_start(xq_sb[:], nc.d["xq"].rearrange("(g p) q -> p g q", p=P))
        nc.sync.dma_start(xk_sb[:], nc.d["xk"].rearrange("(g p) q -> p g q", p=P))
        nc.sync.dma_start(wqT_sb[:], nc.d["wqT"].rearrange("(g p) c -> p g c", p=P))
        nc.sync.dma_start(wkT_sb[:], nc.d["wkT"].rearrange("(g p) c -> p g c", p=P))
        nc.sync.dma_start(wvT_sb[:], nc.d["wvT"].rearrange("(g p) c -> p g c", p=P))
        nc.sync.dma_start(woT_sb[:], nc.d["woT"][:, :])
        nc.sync.dma_start(bq_sb[:], nc.d["bq"][:, :])
        nc.sync.dma_start(bo2_sb[:], nc.d["bo2"].rearrange("(g p) o -> p (g o)", p=P))

        # ---- casts (all on DVE so matmul operands share one semaphore) ----
        nc.vector.tensor_copy(wqT_r[:], wqT_sb[:])
        nc.vector.tensor_copy(wkT_r[:], wkT_sb[:])
        nc.vector.tensor_copy(wvT_bf[:], wvT_sb[:])
        nc.vector.tensor_copy(woT_bf[:], woT_sb[:])
        nc.vector.tensor_copy(xk_r[:], xk_sb[:])
        nc.vector.tensor_copy(xk_bf[:], xk_sb[:])
        nc.vector.tensor_copy(xq_r[:], xq_sb[:])
        # residual base: xq + bo' (per-partition scalar per channel group)
        for g in range(NG):
            nc.vector.tensor_scalar(
                out=xq_sb[:, g, :],
                in0=xq_sb[:, g, :],
                scalar1=bo2_sb[:, g : g + 1],
                scalar2=None,
                op0=mybir.AluOpType.add,
            )

        # ---- projections ----
        # q = wq @ xq + bq : [CH, NQ] in fp32r
        for n in range(NQ // 512):
            pq = ph1ps.tile([P, 512], F32, tag="pq")
            for g in range(NG):
                nc.tensor.matmul(
                    pq[:],
                    wqT_r[:, g, :],
                    xq_r[:, g, n * 512 : (n + 1) * 512],
                    start=(g == 0),
                    stop=(g == NG - 1),
                )
            nc.vector.tensor_scalar(
                out=qr[:, n * 512 : (n + 1) * 512],
                in0=pq[:],
                scalar1=bq_sb[:, 0:1],
                scalar2=None,
                op0=mybir.AluOpType.add,
            )
        # k = wk @ xk : [CH, NK] in fp32r (bk dropped)
        for n in range(NK // 512):
            pk = ph1ps.tile([P, 512], F32, tag="pk")
            for g in range(NG):
                nc.tensor.matmul(
                    pk[:],
                    wkT_r[:, g, :],
                    xk_r[:, g, n * 512 : (n + 1) * 512],
                    start=(g == 0),
                    stop=(g == NG - 1),
                )
            nc.vector.tensor_copy(kr[:, n * 512 : (n + 1) * 512], pk[:])
        # vT[k, c] = xk^T wv^T : 32 tiles [P, CH] in bf16 (bv folded into bo')
        for kc in range(NKC):
            pv = ph1ps.tile([P, CH], F32, tag="pv")
            for g in range(NG):
                nc.tensor.matmul(
                    pv[:],
                    xk_bf[:, g, kc * P : (kc + 1) * P],
                    wvT_bf[:, g, :],
                    start=(g == 0),
                    stop=(g == NG - 1),
                )
            nc.vector.tensor_copy(vt[kc][:, 0:CH], pv[:])

    # ---- main attention loop ----
    et_pool = ctx.enter_context(tc.tile_pool(name="et", bufs=2 * NGRP))
    pe_pool = ctx.enter_context(tc.tile_pool(name="pe", bufs=1, space="PSUM"))
    av_pool = ctx.enter_context(tc.tile_pool(name="av", bufs=2, space="PSUM"))
    tr_pool = ctx.enter_context(tc.tile_pool(name="tr", bufs=1, space="PSUM"))
    py_pool = ctx.enter_context(tc.tile_pool(name="py", bufs=1, space="PSUM"))
    sm_pool = ctx.enter_context(tc.tile_pool(name="sm", bufs=3))

    for jq in range(NCHUNK):
        qs = jq * QC
        # energy (transposed) + exp: eT[k, q] for this q-chunk
        et_tiles = []
        for gp in range(NGRP):
            pe = pe_pool.tile([P, GRP * QC], F32)
            for i in range(GRP):
                kc = gp * GRP + i
                nc.tensor.matmul(
                    pe[:, i * QC : (i + 1) * QC],
                    kr[:, kc * P : (kc + 1) * P],
                    qr[:, qs : qs + QC],
                    start=True,
                    stop=True,
                )
            et = et_pool.tile([P, GRP * QC], AVDT, tag="et")
            nc.scalar.activation(et[:], pe[:], AF.Exp)
            et_tiles.append(et)
        # attention @ v, with fused denominator in column CH
        for qt in range(NQT):
            pav = av_pool.tile([P, NAV], F32)
            for kc in range(NKC):
                gp, i = divmod(kc, GRP)
                lhs = et_tiles[gp][:, i * QC + qt * P : i * QC + qt * P + P]
                nc.tensor.matmul(
                    pav[:],
                    lhs,
                    vt[kc][:, 0:NAV],
                    start=(kc == 0),
                    stop=(kc == NKC - 1),
                )
            recip = sm_pool.tile([P, 1], F32, tag="recip")
            nc.vector.reciprocal(recip[:], pav[:, CH : CH + 1])
            outT = sm_pool.tile([P, P], AVDT, tag="outT")
            nc.scalar.mul(outT[:], pav[:, 0:CH], recip[:])
            ptr = tr_pool.tile([P, P], AVDT)
            nc.tensor.transpose(ptr[:], outT[:], ident[:])
            outc = sm_pool.tile([P, P], AVDT, tag="outc")
            nc.vector.tensor_copy(outc[:], ptr[:])
            # y = wo @ out + (xq + bo')
            py = py_pool.tile([P, NG * P], F32)
            for g in range(NG):
                nc.tensor.matmul(
                    py[:, g * P : (g + 1) * P],
                    woT_bf[:, g * P : (g + 1) * P],
                    outc[:],
                    start=True,
                    stop=True,
                )
            qoff = qs + qt * P
            for g in range(NG):
                nc.vector.tensor_add(
                    y_sb[:, g, qoff : qoff + P],
                    py[:, g * P : (g + 1) * P],
                    xq_sb[:, g, qoff : qoff + P],
                )

    nc.sync.dma_start(nc.d["y"].rearrange("(g p) q -> p g q", p=P), y_sb[:])


class _DramTensors:
    def __init__(self, nc):
        self._aps = {}
        self.nc = nc

    def add(self, name, shape, dtype, kind):
        self._aps[name] = self.nc.dram_tensor(name, shape, dtype, kind=kind).ap()

    def __getitem__(self, name):
        return self._aps[name]


_PROGRAM = None


def _build_program():
    global _PROGRAM
    if _PROGRAM is not None:
        return _PROGRAM
    nc = bass.Bass("TRN2", debug=False, num_devices=8)
    d = _DramTensors(nc)
    nc.d = d
    d.add("xq", [CIN, NQ], F32, "ExternalInput")
    d.add("xk", [CIN, NK], F32, "ExternalInput")
    d.add("wqT", [CIN, CH], F32, "ExternalInput")
    d.add("wkT", [CIN, CH], F32, "ExternalInput")
    d.add("wvT", [CIN, CH], F32, "ExternalInput")
    d.add("woT", [CH, CIN], F32, "ExternalInput")
    d.add("bq", [CH, 1], F32, "ExternalInput")
    d.add("bo2", [CIN, 1], F32, "ExternalInput")
    d.add("y", [CIN, NQ], F32, "ExternalOutput")
    with tile.TileContext(nc) as tc, ExitStack() as ctx:
        _emit(nc, tc, ctx)
    _split_multi_waits(nc)
    _PROGRAM = nc
    return nc


def make_in_maps(inputs):
    """Shard full inputs into per-core input maps (host-side, cheap)."""
    B, C, H, W = 4, 256, 64, 64
    xq = np.ascontiguousarray(np.asarray(inputs["x_query"], np.float32)).reshape(
        B, C, H * W
    )
    xk = np.ascontiguousarray(np.asarray(inputs["x_key"], np.float32)).reshape(
        B, C, H * W
    )
    wq = np.asarray(inputs["wq"], np.float32)
    wk = np.asarray(inputs["wk"], np.float32)
    wv = np.asarray(inputs["wv"], np.float32)
    wo = np.asarray(inputs["wo"], np.float32)
    bq = np.asarray(inputs["bq"], np.float32)
    bo = np.asarray(inputs["bo"], np.float32)
    bv = np.asarray(inputs["bv"], np.float32)
    wqT = np.ascontiguousarray(wq.T)
    wkT = np.ascontiguousarray(wk.T)
    wvT = np.ascontiguousarray(wv.T)
    woT = np.ascontiguousarray(wo.T)
    bo2 = np.ascontiguousarray((bo + wo @ bv).reshape(CIN, 1))
    bq2 = np.ascontiguousarray(bq.reshape(CH, 1))
    in_maps = []
    for core in range(8):
        b, qh = divmod(core, 2)
        in_maps.append(
            {
                "xq": np.ascontiguousarray(xq[b][:, qh * NQ : (qh + 1) * NQ]),
                "xk": xk[b],
                "wqT": wqT,
                "wkT": wkT,
                "wvT": wvT,
                "woT": woT,
                "bq": bq2,
                "bo2": bo2,
            }
        )
    return in_maps


def gather_output(results):
    B, C, H, W = 4, 256, 64, 64
    y = np.empty((B, C, H * W), np.float32)
    for core in range(8):
        b, qh = divmod(core, 2)
        y[b][:, qh * NQ : (qh + 1) * NQ] = results[core]["y"]
    return y.reshape(B, C, H, W)


def kernel(**inputs):
    nc = _build_program()
    in_maps = make_in_maps(inputs)
    res = run_bass_kernel_spmd(nc, in_maps, core_ids=list(range(8)))
    return gather_output(res.results)


if __name__ == "__main__":
    # smoke test with random data
    rng = np.random.default_rng(0)
    B, C, H, W = 4, 256, 64, 64
    Ch = C // 2
    s_in, s_h = 1 / np.sqrt(C), 1 / np.sqrt(Ch)
    inputs = {
        "x_query": rng.standard_normal((B, C, H, W), np.float32),
        "x_key": rng.standard_normal((B, C, H, W), np.float32),
        "wq": rng.uniform(-s_in, s_in, (Ch, C)).astype(np.float32),
        "bq": rng.uniform(-s_in, s_in, (Ch,)).astype(np.float32),
        "wk": rng.uniform(-s_in, s_in, (Ch, C)).astype(np.float32),
        "bk": rng.uniform(-s_in, s_in, (Ch,)).astype(np.float32),
        "wv": rng.uniform(-s_in, s_in, (Ch, C)).astype(np.float32),
        "bv": rng.uniform(-s_in, s_in, (Ch,)).astype(np.float32),
        "wo": rng.uniform(-s_h, s_h, (C, Ch)).astype(np.float32),
        "bo": rng.uniform(-s_h, s_h, (C,)).astype(np.float32),
    }
    y = kernel(**inputs)
    print("kernel output", y.shape, y.dtype, np.abs(y).max())
